# revision 8
# baseline (speedup 1.0000x reference)
"""DPNModel forward for Trainium2: fc6/fc7 GEMMs (75% of FLOPs) run on 8
NeuronCores via Bass/Tile; light stages (small convs, local 50x50 modulated
conv, upsample, softmax) run on host in numpy.

Sharding: fc6 (K=25088, M=4096, N=256) is sharded over output channels
(512/core); its output feeds fc7 (K=4096, M=4096, N=256) sharded over INPUT
channels, so each core consumes exactly the fc6 channels it produced — no
cross-core exchange on device; the 8 fc7 partial sums are added on host.
"""

import sys

sys.path.insert(0, "/opt/trn_rl_repo")

import ml_dtypes
import numpy as np

# ---- static config (mirrors the DPNModel reference) ----
H, W = 128, 128
L = 21
M = 5
K = 50
PAD_LO, PAD_HI = 25, 24
NEG = 0.01

BF16 = ml_dtypes.bfloat16

# GEMM1 (fc6): K1 = 512*49 = 25088 = 196*128, M per core = 512 = 4*128, N = 256
KC1 = 196
MC1 = 4
KHALF = 49  # ktiles per streamed weight chunk (4 chunks per m-tile)
# GEMM2 (fc7): K2 per core = 512 = 4*128, M = 4096 = 32*128, N = 256
KC2 = 4
MC2 = 32
N1 = 256

_DEVICE_FN = None


def _build_device_kernel():
    from contextlib import ExitStack

    import concourse.bass as bass
    import concourse.mybir as mybir

    bf16 = mybir.dt.bfloat16
    f32 = mybir.dt.float32

    nc = bass.Bass()
    a6 = nc.declare_dram_parameter("a6", [128, KC1, N1], bf16, isOutput=False)
    w6 = nc.declare_dram_parameter("w6", [MC1, 128, KC1, 128], bf16, isOutput=False)
    b6 = nc.declare_dram_parameter("b6", [128, MC1], f32, isOutput=False)
    w7 = nc.declare_dram_parameter("w7", [128, KC2, MC2, 128], bf16, isOutput=False)
    out = nc.declare_dram_parameter("out", [MC2, 128, N1], f32, isOutput=True)

    NCH = 4 * MC1  # 16 streamed weight chunks of KHALF ktiles each

    with ExitStack() as ctx:
        a6_t = ctx.enter_context(nc.sbuf_tensor("a6_t", [128, KC1, N1], bf16))
        b6_t = ctx.enter_context(nc.sbuf_tensor("b6_t", [128, MC1], f32))
        w7_t = ctx.enter_context(
            nc.sbuf_tensor("w7_t", [128, KC2, MC2, 128], bf16)
        )
        h6_t = ctx.enter_context(nc.sbuf_tensor("h6_t", [128, KC2, N1], bf16))
        wt = [
            ctx.enter_context(
                nc.sbuf_tensor(f"wt{i}", [128, KHALF, 128], bf16)
            )
            for i in range(2)
        ]
        t0 = ctx.enter_context(nc.sbuf_tensor("t0", [128, N1], f32))
        t1 = ctx.enter_context(nc.sbuf_tensor("t1", [128, N1], f32))
        ot = [
            ctx.enter_context(nc.sbuf_tensor(f"ot{i}", [128, N1], f32))
            for i in range(2)
        ]
        ps1 = [
            ctx.enter_context(nc.psum_tensor(f"ps1_{i}", [128, N1], f32))
            for i in range(2)
        ]
        ps2 = [
            ctx.enter_context(nc.psum_tensor(f"ps2_{i}", [128, N1], f32))
            for i in range(2)
        ]
        insem = ctx.enter_context(nc.semaphore("insem"))
        wsem = ctx.enter_context(nc.semaphore("wsem"))
        csem = ctx.enter_context(nc.semaphore("csem"))
        hsem = ctx.enter_context(nc.semaphore("hsem"))
        gsem = ctx.enter_context(nc.semaphore("gsem"))
        osem = ctx.enter_context(nc.semaphore("osem"))
        dsem = ctx.enter_context(nc.semaphore("dsem"))
        block = ctx.enter_context(nc.Block())

        @block.sync
        def _(sync):
            sync.dma_start(a6_t[:], a6[:]).then_inc(insem, 16)
            sync.dma_start(b6_t[:], b6[:]).then_inc(insem, 16)
            sync.dma_start(w7_t[:], w7[:]).then_inc(insem, 16)
            # stream fc6 weight chunks, double-buffered
            for c in range(NCH):
                m, q = c // 4, c % 4
                if c >= 2:
                    sync.wait_ge(csem, c - 1)
                sync.dma_start(
                    wt[c % 2][:], w6[m, :, q * KHALF : (q + 1) * KHALF, :]
                ).then_inc(wsem, 16)
            # drain fc7 partial tiles to HBM
            for m2 in range(MC2):
                sync.wait_ge(osem, m2 + 1)
                sync.dma_start(out[m2], ot[m2 % 2][:]).then_inc(dsem, 16)
            sync.wait_ge(dsem, 16 * MC2)

        @block.tensor
        def _(tensor):
            tensor.wait_ge(insem, 48)
            # GEMM1: 196 k-tiles accumulated per output m-chunk
            for c in range(NCH):
                m, q = c // 4, c % 4
                tensor.wait_ge(wsem, 16 * (c + 1))
                if q == 0 and m >= 2:
                    tensor.wait_ge(hsem, m - 1)  # ps1[m%2] free
                for kk in range(KHALF):
                    k = q * KHALF + kk
                    mm = tensor.matmul(
                        ps1[m % 2][:],
                        wt[c % 2][:, kk, :],
                        a6_t[:, k, :],
                        start=(k == 0),
                        stop=(k == KC1 - 1),
                    )
                mm.then_inc(csem, 1)
            # GEMM2
            tensor.wait_ge(hsem, MC1)
            for m2 in range(MC2):
                if m2 >= 2:
                    tensor.wait_ge(osem, m2 - 1)  # ps2[m2%2] free
                for k in range(KC2):
                    mm = tensor.matmul(
                        ps2[m2 % 2][:],
                        w7_t[:, k, m2, :],
                        h6_t[:, k, :],
                        start=(k == 0),
                        stop=(k == KC2 - 1),
                    )
                mm.then_inc(gsem, 1)

        @block.vector
        def _(vector):
            vector.wait_ge(insem, 32)  # b6 loaded
            for m in range(MC1):
                vector.wait_ge(csem, 4 * (m + 1))  # group m matmuls complete
                vector.tensor_scalar_add(t0[:], ps1[m % 2][:], b6_t[:, m : m + 1])
                vector.tensor_scalar_mul(t1[:], t0[:], NEG)
                vector.tensor_tensor(
                    h6_t[:, m, :], t0[:], t1[:], mybir.AluOpType.max
                ).then_inc(hsem, 1)
            for m2 in range(MC2):
                vector.wait_ge(gsem, m2 + 1)
                if m2 >= 2:
                    vector.wait_ge(dsem, 16 * (m2 - 1))  # ot[m2%2] drained
                vector.tensor_copy(ot[m2 % 2][:], ps2[m2 % 2][:]).then_inc(
                    osem, 1
                )

    return nc


def _run_device(a6_np, w6_full, b6_full, w7_full):
    """a6_np [25088,256] f32; w6_full [25088,4096]; b6_full [4096]; w7_full
    [4096,4096] (O,I). Returns fc7 pre-activation interior [4096, 256] f32."""
    global _DEVICE_FN
    from concourse.bass_utils import run_bass_kernel_spmd

    if _DEVICE_FN is None:
        _DEVICE_FN = _build_device_kernel()
    nc = _DEVICE_FN

    a6_bf = (
        a6_np.astype(BF16).reshape(KC1, 128, N1).transpose(1, 0, 2).copy()
    )  # [128, KC1, N1]
    in_maps = []
    for i in range(8):
        sl = slice(512 * i, 512 * (i + 1))
        w6s = w6_full[:, sl]  # [25088, 512]
        w6t = (
            w6s.astype(BF16)
            .reshape(KC1, 128, MC1, 128)
            .transpose(2, 1, 0, 3)
            .copy()
        )  # [MC1, 128, KC1, 128]
        b6t = b6_full[sl].astype(np.float32).reshape(MC1, 128).T.copy()  # [128,MC1]
        w7sT = w7_full[:, sl].T  # [512, 4096]
        w7t = (
            w7sT.astype(BF16)
            .reshape(KC2, 128, MC2, 128)
            .transpose(1, 0, 2, 3)
            .copy()
        )  # [128, KC2, MC2, 128]
        in_maps.append({"a6": a6_bf, "w6": w6t, "b6": b6t, "w7": w7t})

    res = run_bass_kernel_spmd(nc, in_maps, list(range(8)))
    total = np.zeros((4096, N1), np.float32)
    for r in res.results:
        total += np.asarray(r["out"], np.float32).reshape(4096, N1)
    return total


# ---------------- host-side numpy stages ----------------


def _leaky(x):
    return np.where(x >= 0, x, NEG * x)


def _conv2d(x, w, b, dil, pad):
    # x [1,C,H,W], w [O,C,k,k]; returns [1,O,Ho,Wo]
    C, Hi, Wi = x.shape[1], x.shape[2], x.shape[3]
    O, k = w.shape[0], w.shape[2]
    Ho = Hi + 2 * pad - dil * (k - 1)
    Wo = Wi + 2 * pad - dil * (k - 1)
    xp = np.pad(x, ((0, 0), (0, 0), (pad, pad), (pad, pad)))
    out = np.zeros((1, O, Ho, Wo), np.float32)
    xr = xp.reshape(C, xp.shape[2], xp.shape[3])
    for kh in range(k):
        for kw in range(k):
            xs = xr[:, kh * dil : kh * dil + Ho, kw * dil : kw * dil + Wo]
            out[0] += np.einsum(
                "oc,chw->ohw", w[:, :, kh, kw], xs, optimize=True
            )
    return out + b[None, :, None, None]


def _pool(x):
    B, C, Hi, Wi = x.shape
    return x.reshape(B, C, Hi // 2, 2, Wi // 2, 2).max(axis=(3, 5))


def _upsample_ac(x, out_h, out_w):
    B, C, h, w = x.shape

    def idx(out, inn):
        c = np.arange(out, dtype=np.float32) * np.float32((inn - 1) / (out - 1))
        i0 = np.floor(c).astype(np.int32)
        i1 = np.minimum(i0 + 1, inn - 1)
        return i0, i1, (c - i0.astype(np.float32))

    r0, r1, rf = idx(out_h, h)
    c0, c1, cf = idx(out_w, w)
    xr = x[:, :, r0, :] * (1 - rf)[None, None, :, None] + x[:, :, r1, :] * rf[
        None, None, :, None
    ]
    return xr[:, :, :, c0] * (1 - cf) + xr[:, :, :, c1] * cf


def _sigmoid(x):
    return 1.0 / (1.0 + np.exp(-x))


def kernel(
    input_tensor,
    backbone_params,
    local_w,
    local_b,
    lin1_w,
    lin1_b,
    glob_w,
    glob_b,
    lin2_w,
    lin2_b,
):
    x = np.asarray(input_tensor, np.float32)
    bp = [(np.asarray(w, np.float32), np.asarray(b, np.float32)) for (w, b) in backbone_params]
    local_w = np.asarray(local_w, np.float32)
    local_b = np.asarray(local_b, np.float32)
    lin1_w = np.asarray(lin1_w, np.float32)
    lin1_b = np.asarray(lin1_b, np.float32)
    glob_w = np.asarray(glob_w, np.float32)
    glob_b = np.asarray(glob_b, np.float32)
    lin2_w = np.asarray(lin2_w, np.float32)
    lin2_b = np.asarray(lin2_b, np.float32)

    # ---- backbone conv1-5 on host (12.7 GFLOP, BLAS) ----
    # ops: c,c,P,c,c,P,c,c,c,P,c,c,c,c(d2),c(d2),c(d2) then fc6,fc7,final
    plan = [
        ("c", 0, 1, 1), ("c", 1, 1, 1), ("p",),
        ("c", 2, 1, 1), ("c", 3, 1, 1), ("p",),
        ("c", 4, 1, 1), ("c", 5, 1, 1), ("c", 6, 1, 1), ("p",),
        ("c", 7, 1, 1), ("c", 8, 1, 1), ("c", 9, 1, 1),
        ("c", 10, 2, 2), ("c", 11, 2, 2), ("c", 12, 2, 2),
    ]
    h = x
    for step in plan:
        if step[0] == "p":
            h = _pool(h)
        else:
            _, ci, d, p = step
            w, b = bp[ci]
            h = _leaky(_conv2d(h, w, b, d, p))
    h5 = h[0]  # [512, 16, 16]

    # ---- im2col for fc6 (k=7, dil=4, pad=12) ----
    hp = np.pad(h5, ((0, 0), (12, 12), (12, 12)))  # [512, 40, 40]
    A6 = np.empty((512, 49, 256), np.float32)
    for kh in range(7):
        for kw in range(7):
            A6[:, kh * 7 + kw, :] = hp[
                :, 4 * kh : 4 * kh + 16, 4 * kw : 4 * kw + 16
            ].reshape(512, 256)
    A6 = A6.reshape(25088, 256)

    w6_, b6_ = bp[13]
    w7_, b7_ = bp[14]
    wf_, bf_ = bp[15]
    W6 = w6_.reshape(4096, 25088).T  # [K, M]
    W7 = w7_.reshape(4096, 4096)  # [O, I]

    # ---- device: fc6 + leaky + fc7 partials on 8 NeuronCores ----
    fc7_pre = _run_device(A6, W6, b6_, W7)  # [4096, 256] (no bias)

    # fc7 output is 18x18 (k=1, pad=1): border ring = bias only
    h7 = np.empty((4096, 18, 18), np.float32)
    h7[:] = _leaky(b7_)[:, None, None]
    h7[:, 1:17, 1:17] = _leaky(fc7_pre + b7_[:, None]).reshape(4096, 16, 16)

    # final 1x1 conv -> [21, 18, 18]
    hf = _leaky(
        wf_.reshape(L, 4096) @ h7.reshape(4096, 324) + bf_[:, None]
    ).reshape(1, L, 18, 18)

    unary = _sigmoid(_upsample_ac(hf, H, W))  # [1, L, 128, 128]

    # ---- locally-connected 50x50 conv modulated by color distance ----
    pad_spec = ((0, 0), (0, 0), (PAD_LO, PAD_HI), (PAD_LO, PAD_HI))
    img_pad = np.pad(x, pad_spec)[0]  # [3, 177, 177]
    un_pad = np.pad(unary, pad_spec)[0]  # [L, 177, 177]
    swv = np.lib.stride_tricks.sliding_window_view
    acc = np.zeros((L, H, W), np.float32)
    x0 = x[0]  # [3, H, W]
    for kh in range(K):
        ir = img_pad[:, kh : kh + H, :]  # [3, H, W+49]
        ur = un_pad[:, kh : kh + H, :]  # [L, H, W+49]
        i_s = swv(ir, K, axis=2)  # [3, H, W, K]
        u_s = swv(ur, K, axis=2)  # [L, H, W, K]
        dcol = ((x0[..., None] - i_s) ** 2).sum(axis=0)  # [H, W, K]
        m = dcol * local_w[:, :, kh, :]  # [H, W, K]
        acc += np.einsum("lhwk,hwk->lhw", u_s, m, optimize=True)
    local_out = acc + local_b[:, None, None]  # [L, H, W]

    # ---- lin1 -> glob 9x9 conv -> lin2 -> min over mixtures -> softmax ----
    t = np.einsum("lhw,ol->ohw", local_out, lin1_w) + lin1_b[:, None, None]
    g = _conv2d(t[None], glob_w, glob_b, 1, 4)  # [1, L*M, 128, 128]
    g = np.einsum("chw,oc->ohw", g[0], lin2_w) + lin2_b[:, None, None]
    smooth = g.reshape(M, L, H, W).min(axis=0)  # [L, H, W]
    z = np.log(unary[0]) - smooth
    z = z - z.max(axis=0, keepdims=True)
    ez = np.exp(z)
    return (ez / ez.sum(axis=0, keepdims=True))[None].astype(np.float32)
